# revision 2
# baseline (speedup 1.0000x reference)
"""Masked multi-head self-attention for Trainium2, SPMD over 8 NeuronCores.

Sharding: core c handles batch c//2, query-half c%2 (1024 of 2048 query rows).
The same Bass program runs on every core; odd cores get their q / mask inputs
rotated along the key axis so that "my" query rows are always tokens [0, 1024)
(attention sums are invariant to a consistent permutation of the key axis).

Per-core dataflow (all matmul inputs bf16, fp32 PSUM accumulation):
  x^T   via bf16 convert + DMA-xbar transpose
  Q^T/K^T (head-pair packed) and V (token-major, with a ones column for the
        softmax denominator) via PE projections
  S^T   = K @ Q^T per (head, key-tile) in PSUM
  U     = exp(0.125 * S^T) on ACT (PSUM -> SBUF bf16), masked by a
        pre-transposed (1-mask) via one DVE multiply
  headsT/denominator via PE (U moving, [V | 1] stationary)
  normalize by broadcast reciprocal (PE rank-1 broadcast), output projection
        accumulated over heads in PSUM.
"""

import sys

sys.path.insert(0, "/opt/trn_rl_repo")

import numpy as np

import concourse.bass as bass  # noqa: F401  (bass types used indirectly)
import concourse.tile as tile
from concourse import bacc, mybir
from concourse.bass_utils import run_bass_kernel_spmd

F32 = mybir.dt.float32
BF16 = mybir.dt.bfloat16
I32 = mybir.dt.int32
EXP = mybir.ActivationFunctionType.Exp
MUL = mybir.AluOpType.mult
ADD = mybir.AluOpType.add

B, N, D, H, DK = 4, 2048, 512, 8, 64
NQ = N // 2          # query rows per core
NORM = 1.0 / 8.0     # 1/sqrt(DK)
NTT = N // 128       # token tiles (16)
NFC = D // 128       # feature chunks (4)
NHP = H // 2         # head pairs (4)
NGT = N // 128       # key tiles (16)
NQT = NQ // 128      # query tiles per core (8)
NCORES = 8

_CACHE = {}


def _build():
    if "nc" in _CACHE:
        return _CACHE["nc"]
    nc = bacc.Bacc("TRN2", target_bir_lowering=False, debug=False,
                   num_devices=NCORES)
    xq = nc.dram_tensor("xq", [N, D], F32, kind="ExternalInput")
    msk = nc.dram_tensor("mask", [NQ, N], I32, kind="ExternalInput")
    wq = nc.dram_tensor("wq", [D, D], F32, kind="ExternalInput")
    wk = nc.dram_tensor("wk", [D, D], F32, kind="ExternalInput")
    wv = nc.dram_tensor("wv", [D, D], F32, kind="ExternalInput")
    wo = nc.dram_tensor("wo", [DK, H * D], F32, kind="ExternalInput")
    out = nc.dram_tensor("out", [NQ, D], F32, kind="ExternalOutput")

    with tile.TileContext(nc) as tc:
        with tc.tile_pool(name="persist", bufs=1) as P:
            # K^T / Q^T packed two heads per tile: partition = (head%2)*64 + k
            kt = P.tile([128, NHP, N], BF16)
            qt_ = P.tile([128, NHP, NQ], BF16)
            # V per key-tile, per head; column DK holds ones (denominator row)
            v_ = P.tile([128, NGT, H, DK + 1], BF16)
            # (1 - mask)^T: [g-part, qtile, gchunk, q]
            nmt = P.tile([128, NQT, NGT, 128], BF16)
            # rows 0..63 = headsT, row 64 = denominator (later its reciprocal)
            hts = P.tile([DK + 1, H, NQ], F32)
            wob = P.tile([DK, H * D], BF16)
            ones = P.tile([128, DK], F32)
            nc.vector.memset(ones[:], 1.0)
            nc.vector.memset(v_[:, :, :, DK:DK + 1], 1.0)

            # ---------------- phase A: loads, transposes, projections -------
            with tc.tile_pool(name="pa", bufs=2) as A, \
                 tc.tile_pool(name="paps", bufs=3, space="PSUM") as APs, \
                 tc.tile_pool(name="xtp", bufs=1) as XT:
                xt = XT.tile([128, NTT, NFC, 128], BF16)
                wqb = XT.tile([128, NFC, D], BF16)
                wkb = XT.tile([128, NFC, D], BF16)
                wvb = XT.tile([128, NFC, D], BF16)
                for dram, dst in ((wq, wqb), (wk, wkb), (wv, wvb)):
                    for fc in range(NFC):
                        st = A.tile([128, D], F32, tag="wstage")
                        nc.sync.dma_start(
                            out=st[:], in_=dram[fc * 128:(fc + 1) * 128, :])
                        nc.vector.tensor_copy(dst[:, fc, :], st[:])
                wos = A.tile([DK, H * D], F32, tag="wostage")
                nc.sync.dma_start(out=wos[:], in_=wo[:, :])
                nc.vector.tensor_copy(wob[:], wos[:])

                for tt in range(NTT):
                    xs = A.tile([128, D], F32, tag="xstage")
                    nc.sync.dma_start(
                        out=xs[:], in_=xq[tt * 128:(tt + 1) * 128, :])
                    xb = A.tile([128, D], BF16, tag="xbf")
                    nc.vector.tensor_copy(xb[:], xs[:])
                    nc.sync.dma_start_transpose(out=xt[:, tt, :, :], in_=xb[:])

                for qt in range(NQT):
                    for hf in range(2):
                        ms = A.tile([128, N // 2], I32, tag="mstage")
                        nc.sync.dma_start(
                            out=ms[:],
                            in_=msk[qt * 128:(qt + 1) * 128,
                                    hf * (N // 2):(hf + 1) * (N // 2)])
                        nb = A.tile([128, N // 2], BF16, tag="nbf")
                        # notm = mask * -1 + 1
                        nc.vector.tensor_scalar(nb[:], ms[:], -1.0, 1.0, MUL, ADD)
                        nc.sync.dma_start_transpose(
                            out=nmt[:, qt, hf * 8:(hf + 1) * 8, :], in_=nb[:])

                # K^T (all 2048 tokens) and Q^T (my 1024 rows = tokens 0..1023)
                for hp in range(NHP):
                    for ttg in range(4):
                        ps = APs.tile([128, 512], F32, tag="projps")
                        for fc in range(NFC):
                            nc.tensor.matmul(
                                ps[:],
                                wkb[:, fc, hp * 128:(hp + 1) * 128],
                                xt[:, ttg * 4:(ttg + 1) * 4, fc, :],
                                start=(fc == 0), stop=(fc == NFC - 1))
                        nc.vector.tensor_copy(
                            kt[:, hp, ttg * 512:(ttg + 1) * 512], ps[:])
                    for ttg in range(2):
                        ps = APs.tile([128, 512], F32, tag="projps")
                        for fc in range(NFC):
                            nc.tensor.matmul(
                                ps[:],
                                wqb[:, fc, hp * 128:(hp + 1) * 128],
                                xt[:, ttg * 4:(ttg + 1) * 4, fc, :],
                                start=(fc == 0), stop=(fc == NFC - 1))
                        nc.vector.tensor_copy(
                            qt_[:, hp, ttg * 512:(ttg + 1) * 512], ps[:])
                # V: token-major [g, (h, v)]
                for gt in range(NGT):
                    ps = APs.tile([128, 512], F32, tag="projps")
                    for fc in range(NFC):
                        nc.tensor.matmul(
                            ps[:], xt[:, gt, fc, :], wvb[:, fc, :],
                            start=(fc == 0), stop=(fc == NFC - 1))
                    nc.vector.tensor_copy(
                        v_[:, gt, :, 0:DK],
                        ps.rearrange("p (h v) -> p h v", h=H))

            # ---------------- phase B: scores, exp, mask, attn @ V ----------
            with tc.tile_pool(name="ub", bufs=3) as UB, \
                 tc.tile_pool(name="sps", bufs=2, space="PSUM") as SPs, \
                 tc.tile_pool(name="hvps", bufs=1, space="PSUM") as HVs:
                for hp in range(NHP):
                    hv = [HVs.tile([DK + 1, NQ], F32, tag=f"hv{i}",
                                   name=f"hv{i}")
                          for i in range(2)]
                    for gt in range(NGT):
                        for i in range(2):
                            h = hp * 2 + i
                            s = SPs.tile([128, NQ], F32, tag="sps")
                            for qg in range(2):
                                nc.tensor.matmul(
                                    s[:, qg * 512:(qg + 1) * 512],
                                    kt[i * 64:(i + 1) * 64, hp,
                                       gt * 128:(gt + 1) * 128],
                                    qt_[i * 64:(i + 1) * 64, hp,
                                        qg * 512:(qg + 1) * 512],
                                    start=True, stop=True)
                            u = UB.tile([128, NQ], BF16, tag="u")
                            nc.scalar.activation(u[:], s[:], EXP,
                                                 bias=0.0, scale=NORM)
                            uv = u.rearrange("p (a b) -> p a b", b=128)
                            nc.vector.tensor_mul(uv, uv, nmt[:, :, gt, :])
                            for qg in range(2):
                                nc.tensor.matmul(
                                    hv[i][:, qg * 512:(qg + 1) * 512],
                                    v_[:, gt, h, :],
                                    u[:, qg * 512:(qg + 1) * 512],
                                    start=(gt == 0), stop=(gt == NGT - 1))
                    for i in range(2):
                        nc.vector.tensor_copy(hts[:, hp * 2 + i, :], hv[i][:])

            # ---------------- phase C/D: normalize + output projection ------
            nc.vector.reciprocal(hts[DK:DK + 1, :, :], hts[DK:DK + 1, :, :])
            with tc.tile_pool(name="pd", bufs=2) as DP, \
                 tc.tile_pool(name="pdps", bufs=2, space="PSUM") as DPs, \
                 tc.tile_pool(name="htnp", bufs=1) as HTN:
                htn = HTN.tile([DK, H, NQ], BF16)
                for h in range(H):
                    rb = DPs.tile([DK, NQ], F32, tag="rb")
                    for qg in range(2):
                        nc.tensor.matmul(
                            rb[:, qg * 512:(qg + 1) * 512],
                            ones[DK:DK + 1, :],
                            hts[DK:DK + 1, h, qg * 512:(qg + 1) * 512],
                            start=True, stop=True)
                    nc.vector.tensor_mul(htn[:, h, :], hts[0:DK, h, :], rb[:])
                for qt in range(NQT):
                    po = DPs.tile([128, 512], F32, tag="po")
                    for h in range(H):
                        nc.tensor.matmul(
                            po[:],
                            htn[:, h, qt * 128:(qt + 1) * 128],
                            wob[:, h * D:(h + 1) * D],
                            start=(h == 0), stop=(h == H - 1))
                    ob = DP.tile([128, 512], F32, tag="ob")
                    nc.vector.tensor_copy(ob[:], po[:])
                    nc.sync.dma_start(
                        out=out[qt * 128:(qt + 1) * 128, :], in_=ob[:])

    nc.compile()
    _CACHE["nc"] = nc
    return nc


def kernel(q, mask, W_query, W_key, W_val, W_out):
    q = np.ascontiguousarray(q, dtype=np.float32)
    mask = np.ascontiguousarray(mask, dtype=np.int32)
    # [f, h*64+k] layouts for the projections, [k, h*512+e] for the output
    wq_r = np.ascontiguousarray(
        np.transpose(np.asarray(W_query, np.float32), (1, 0, 2)).reshape(D, D))
    wk_r = np.ascontiguousarray(
        np.transpose(np.asarray(W_key, np.float32), (1, 0, 2)).reshape(D, D))
    wv_r = np.ascontiguousarray(
        np.transpose(np.asarray(W_val, np.float32), (1, 0, 2)).reshape(D, D))
    wo_r = np.ascontiguousarray(
        np.transpose(np.asarray(W_out, np.float32), (1, 0, 2)).reshape(DK, H * D))

    nc = _build()
    in_maps = []
    for c in range(NCORES):
        b, qh = c // 2, c % 2
        xq_c = q[b]
        m_c = mask[b, qh * NQ:(qh + 1) * NQ, :]
        if qh:
            # rotate the key axis so this core's queries are tokens [0, NQ)
            xq_c = np.roll(xq_c, -NQ, axis=0)
            m_c = np.roll(m_c, -NQ, axis=1)
        in_maps.append({
            "xq": np.ascontiguousarray(xq_c),
            "mask": np.ascontiguousarray(m_c),
            "wq": wq_r, "wk": wk_r, "wv": wv_r, "wo": wo_r,
        })
    res = run_bass_kernel_spmd(nc, in_maps, core_ids=list(range(NCORES)))
    output = np.empty((B, N, D), np.float32)
    for c in range(NCORES):
        b, qh = c // 2, c % 2
        output[b, qh * NQ:(qh + 1) * NQ, :] = res.results[c]["out"]
    return output


# revision 6
# speedup vs baseline: 1.3793x; 1.3793x over previous
"""Masked multi-head self-attention for Trainium2, SPMD over 8 NeuronCores.

Sharding: core c handles batch c//2, query-half c%2 (1024 of 2048 query rows).
The same Bass program runs on every core; odd cores get their inputs rotated
along the key axis so that "my" query rows are always tokens [0, 1024)
(attention sums are invariant to a consistent permutation of the key axis).

Host supplies x^T (features-major q) and (1-mask)^T in bf16, so the device
does no transposes. Per-core dataflow:
  Q^T/K^T (head-pair packed, fp32r) and V (token-major bf16, with a ones
        column for the softmax denominator) via PE projections from x^T
  S^T   = K @ Q^T per (head, key-tile) into fp32 PSUM (fp32r operands)
  U     = exp(0.125 * S^T) on ACT (PSUM -> SBUF bf16), masked by (1-mask)^T
        via one DVE multiply
  headsT + denominator via PE (U moving, [V | 1] stationary, bf16)
  normalize via reciprocal + GPSIMD partition-broadcast + DVE multiply,
  output projection (fp32r) accumulated over heads in PSUM.
"""

import sys

sys.path.insert(0, "/opt/trn_rl_repo")

import ml_dtypes
import numpy as np

import concourse.bass as bass  # noqa: F401
import concourse.tile as tile
from concourse import bacc, mybir
from concourse.bass_utils import run_bass_kernel_spmd

F32 = mybir.dt.float32
F32R = mybir.dt.float32r
BF16 = mybir.dt.bfloat16
EXP = mybir.ActivationFunctionType.Exp

B, N, D, H, DK = 4, 2048, 512, 8, 64
NQ = N // 2          # query rows per core
NORM = 1.0 / 8.0     # 1/sqrt(DK)
NFC = D // 128       # feature chunks (4)
NHP = H // 2         # head pairs (4)
NGT = N // 128       # key tiles (16)
NQT = NQ // 128      # query tiles per core (8)
NCORES = 8

_CACHE = {}


def _build():
    if "nc" in _CACHE:
        return _CACHE["nc"]
    nc = bacc.Bacc("TRN2", target_bir_lowering=False, debug=False,
                   num_devices=NCORES)
    xqt = nc.dram_tensor("xqt", [D, N], F32, kind="ExternalInput")
    nmtd = nc.dram_tensor("nmt", [N, NQ], BF16, kind="ExternalInput")
    wq = nc.dram_tensor("wq", [D, D], F32, kind="ExternalInput")
    wk = nc.dram_tensor("wk", [D, D], F32, kind="ExternalInput")
    wv = nc.dram_tensor("wv", [D, D], F32, kind="ExternalInput")
    wo = nc.dram_tensor("wo", [DK, H * D], F32, kind="ExternalInput")
    out = nc.dram_tensor("out", [NQ, D], F32, kind="ExternalOutput")

    with tile.TileContext(nc) as tc:
        with tc.tile_pool(name="persist", bufs=1) as P:
            kt = P.tile([128, NHP, N], F32R)    # K^T two heads per tile
            qt_ = P.tile([128, NHP, NQ], F32R)  # Q^T two heads per tile
            v_ = P.tile([128, NGT, H, DK + 1], BF16)  # V | ones
            nmt = P.tile([128, NGT, NQ], BF16)        # (1-mask)^T
            nc.vector.memset(v_[:, :, :, DK:DK + 1], 1.0)
            nc.sync.dma_start(
                out=nmt[:],
                in_=nmtd.rearrange("(gc p) q -> p gc q", p=128))

            # ---------------- phase A: loads + projections ----------------
            with tc.tile_pool(name="paps", bufs=3, space="PSUM") as APs, \
                 tc.tile_pool(name="xtp", bufs=1) as XT:
                xt = XT.tile([128, NFC, N], F32R)
                wqb = XT.tile([128, NFC, D], F32R)
                wkb = XT.tile([128, NFC, D], F32R)
                wvb = XT.tile([128, NFC, D], F32R)
                for fc in range(NFC):
                    nc.sync.dma_start(
                        out=xt[:, fc, :],
                        in_=xqt[fc * 128:(fc + 1) * 128, :].bitcast(F32R))
                    for dram, dst in ((wq, wqb), (wk, wkb), (wv, wvb)):
                        nc.sync.dma_start(
                            out=dst[:, fc, :],
                            in_=dram[fc * 128:(fc + 1) * 128, :].bitcast(F32R))

                for hp in range(NHP):
                    for ttg in range(4):
                        ps = APs.tile([128, 512], F32, tag="projps")
                        for fc in range(NFC):
                            nc.tensor.matmul(
                                ps[:],
                                wkb[:, fc, hp * 128:(hp + 1) * 128],
                                xt[:, fc, ttg * 512:(ttg + 1) * 512],
                                start=(fc == 0), stop=(fc == NFC - 1))
                        nc.vector.tensor_copy(
                            kt[:, hp, ttg * 512:(ttg + 1) * 512], ps[:])
                    for ttg in range(2):
                        ps = APs.tile([128, 512], F32, tag="projps")
                        for fc in range(NFC):
                            nc.tensor.matmul(
                                ps[:],
                                wqb[:, fc, hp * 128:(hp + 1) * 128],
                                xt[:, fc, ttg * 512:(ttg + 1) * 512],
                                start=(fc == 0), stop=(fc == NFC - 1))
                        nc.vector.tensor_copy(
                            qt_[:, hp, ttg * 512:(ttg + 1) * 512], ps[:])
                for gt in range(NGT):
                    ps = APs.tile([128, 512], F32, tag="projps")
                    for fc in range(NFC):
                        nc.tensor.matmul(
                            ps[:],
                            xt[:, fc, gt * 128:(gt + 1) * 128],
                            wvb[:, fc, :],
                            start=(fc == 0), stop=(fc == NFC - 1))
                    nc.vector.tensor_copy(
                        v_[:, gt, :, 0:DK],
                        ps.rearrange("p (h v) -> p h v", h=H))

            # rows 0..63 headsT, row 64 denominator; dsum = recip-ready rows
            with tc.tile_pool(name="late", bufs=1) as L:
                hts = L.tile([DK + 1, H, NQ], F32)
                dsum = L.tile([1, H, NQ], F32)
                wob = L.tile([DK, H * D], F32R)
                nc.sync.dma_start(out=wob[:], in_=wo[:, :].bitcast(F32R))

                # ---------------- phase B ----------------
                with tc.tile_pool(name="ub", bufs=3) as UB, \
                     tc.tile_pool(name="spsp", bufs=2, space="PSUM") as SPs, \
                     tc.tile_pool(name="hvp", bufs=1, space="PSUM") as HVs:
                    for hp in range(NHP):
                        hv = [HVs.tile([DK + 1, NQ], F32, tag=f"hv{i}",
                                       name=f"hv{i}") for i in range(2)]
                        for gt in range(NGT):
                            for i in range(2):
                                h = hp * 2 + i
                                s = SPs.tile([128, NQ], F32, tag="sps")
                                for qg in range(2):
                                    nc.tensor.matmul(
                                        s[:, qg * 512:(qg + 1) * 512],
                                        kt[i * 64:(i + 1) * 64, hp,
                                           gt * 128:(gt + 1) * 128],
                                        qt_[i * 64:(i + 1) * 64, hp,
                                            qg * 512:(qg + 1) * 512],
                                        start=True, stop=True)
                                u = UB.tile([128, NQ], BF16, tag="u")
                                nc.scalar.activation(u[:], s[:], EXP,
                                                     bias=0.0, scale=NORM)
                                nc.vector.tensor_mul(u[:], u[:], nmt[:, gt, :])
                                for qg in range(2):
                                    nc.tensor.matmul(
                                        hv[i][:, qg * 512:(qg + 1) * 512],
                                        v_[:, gt, h, :],
                                        u[:, qg * 512:(qg + 1) * 512],
                                        start=(gt == 0), stop=(gt == NGT - 1))
                        for i in range(2):
                            h = hp * 2 + i
                            nc.vector.tensor_copy(hts[:, h, :], hv[i][:])
                            nc.scalar.copy(dsum[0:1, h, :],
                                           hts[DK:DK + 1, h, :])

                # ---------------- phase C/D (two query-half passes) --------
                nc.vector.reciprocal(dsum[:], dsum[:])
                with tc.tile_pool(name="pd", bufs=2) as DP, \
                     tc.tile_pool(name="pdps", bufs=2, space="PSUM") as DPs, \
                     tc.tile_pool(name="htnp", bufs=1) as HTN:
                    HQ = NQ // 2
                    for half in range(2):
                        htn = HTN.tile([DK, H, HQ], F32R, tag="htn")
                        for h in range(H):
                            rinvb = DP.tile([DK, HQ], F32, tag="rinvb")
                            nc.gpsimd.partition_broadcast(
                                rinvb[:],
                                dsum[0:1, h, half * HQ:(half + 1) * HQ])
                            nc.vector.tensor_mul(
                                htn[:, h, :],
                                hts[0:DK, h, half * HQ:(half + 1) * HQ],
                                rinvb[:])
                        for qq in range(HQ // 128):
                            qt = half * (HQ // 128) + qq
                            po = DPs.tile([128, 512], F32, tag="po")
                            for h in range(H):
                                nc.tensor.matmul(
                                    po[:],
                                    htn[:, h, qq * 128:(qq + 1) * 128],
                                    wob[:, h * D:(h + 1) * D],
                                    start=(h == 0), stop=(h == H - 1))
                            ob = DP.tile([128, 512], F32, tag="ob")
                            nc.vector.tensor_copy(ob[:], po[:])
                            nc.sync.dma_start(
                                out=out[qt * 128:(qt + 1) * 128, :], in_=ob[:])

    nc.compile()
    _CACHE["nc"] = nc
    return nc


def kernel(q, mask, W_query, W_key, W_val, W_out):
    q = np.asarray(q, dtype=np.float32)
    mask = np.asarray(mask, dtype=np.int32)
    # [f, h*64+k] layouts for the projections, [k, h*512+e] for the output
    wq_r = np.ascontiguousarray(
        np.transpose(np.asarray(W_query, np.float32), (1, 0, 2)).reshape(D, D))
    wk_r = np.ascontiguousarray(
        np.transpose(np.asarray(W_key, np.float32), (1, 0, 2)).reshape(D, D))
    wv_r = np.ascontiguousarray(
        np.transpose(np.asarray(W_val, np.float32), (1, 0, 2)).reshape(D, D))
    wo_r = np.ascontiguousarray(
        np.transpose(np.asarray(W_out, np.float32), (1, 0, 2)).reshape(DK, H * D))

    nc = _build()
    in_maps = []
    for c in range(NCORES):
        b, qh = c // 2, c % 2
        xqt_c = q[b].T                                      # (D, N)
        nmt_c = 1.0 - mask[b, qh * NQ:(qh + 1) * NQ, :].T   # (N, NQ)
        if qh:
            # rotate the key axis so this core's queries are tokens [0, NQ)
            xqt_c = np.roll(xqt_c, -NQ, axis=1)
            nmt_c = np.roll(nmt_c, -NQ, axis=0)
        in_maps.append({
            "xqt": np.ascontiguousarray(xqt_c),
            "nmt": np.ascontiguousarray(nmt_c.astype(ml_dtypes.bfloat16)),
            "wq": wq_r, "wk": wk_r, "wv": wv_r, "wo": wo_r,
        })
    res = run_bass_kernel_spmd(nc, in_maps, core_ids=list(range(NCORES)))
    output = np.empty((B, N, D), np.float32)
    for c in range(NCORES):
        b, qh = c // 2, c % 2
        output[b, qh * NQ:(qh + 1) * NQ, :] = res.results[c]["out"]
    return output
